# revision 1
# baseline (speedup 1.0000x reference)
"""GCN layer kernel for Trainium2, 8-core row-parallel.

Computes out = (adj * mask + I) @ (x @ W^T) for N=8192, C_in=C_out=128.

Sharding: adj/mask row-blocks of 1024 across 8 cores; x, W replicated.
Per core pipeline (heavy matmul work in fp32r = FP22-truncated fp32,
~1e-4 relative error, single-pass on the PE):
  - h = x @ W^T on-chip; x tiles transposed via regular identity matmuls
  - adj/mask row-block streamed in 1MB chunks, multiplied on DVE,
    product tiles transposed via regular identity matmuls on the PE
    (counts as PE-busy, keeps the HAM clock gate warm), PSUM->SBUF copies
    on ACT, then fp32r matmuls (stationary = h k-tile, moving = A^T
    [128k, 512m]) accumulate out^T in PSUM
  - self-loop +h fused into the finalize add after back-transposing out^T
  - loads stream on the SP DMA queue; x/out use the ACT queue so finalize
    writes never block the load FIFO
"""

import numpy as np
from contextlib import ExitStack

from concourse import bass, bacc, tile, mybir
from concourse import masks
from concourse.bass_utils import run_bass_kernel_spmd

N = 8192
C = 128
NCORES = 8
R = N // NCORES          # 1024 rows per core
M_BLK = 512              # psum accumulation block (free dim of main matmul)
NBLK = R // M_BLK        # 2 m-blocks per core
S = M_BLK // 128         # 4 slabs of 128 rows per m-block
KQ = 1024                # k-chunk width per DMA iteration
NQ = N // KQ             # 8 k-chunks
KT = KQ // 128           # 8 k-tiles per chunk
NKT = N // 128           # 64 k-tiles total

F32 = mybir.dt.float32
F32R = mybir.dt.float32r


def build_program():
    nc = bacc.Bacc("TRN2", target_bir_lowering=False, debug=False, num_devices=NCORES)

    adj_d = nc.dram_tensor("adj", [R, N], F32, kind="ExternalInput").ap()
    mask_d = nc.dram_tensor("mask", [R, N], F32, kind="ExternalInput").ap()
    x_d = nc.dram_tensor("x", [N, C], F32, kind="ExternalInput").ap()
    xo_d = nc.dram_tensor("x_own", [R, C], F32, kind="ExternalInput").ap()
    w_d = nc.dram_tensor("w", [C, C], F32, kind="ExternalInput").ap()
    out_d = nc.dram_tensor("out", [R, C], F32, kind="ExternalOutput").ap()

    with tile.TileContext(nc) as tc, ExitStack() as ctx:
        const_pool = ctx.enter_context(tc.tile_pool(name="const", bufs=1))
        xr_pool = ctx.enter_context(tc.tile_pool(name="xr", bufs=2))
        xt_pool = ctx.enter_context(tc.tile_pool(name="xt", bufs=3))
        h_pool = ctx.enter_context(tc.tile_pool(name="h", bufs=1))
        adj_pool = ctx.enter_context(tc.tile_pool(name="adj", bufs=8))
        mask_pool = ctx.enter_context(tc.tile_pool(name="mask", bufs=8))
        prod_pool = ctx.enter_context(tc.tile_pool(name="prod", bufs=8))
        at_pool = ctx.enter_context(tc.tile_pool(name="at", bufs=8))
        fin_pool = ctx.enter_context(tc.tile_pool(name="fin", bufs=2))
        psum_acc = ctx.enter_context(tc.tile_pool(name="pacc", bufs=2, space="PSUM"))
        psum_tr = ctx.enter_context(tc.tile_pool(name="ptr", bufs=3, space="PSUM"))
        psum_misc = ctx.enter_context(tc.tile_pool(name="pmisc", bufs=2, space="PSUM"))
        psum_fin = ctx.enter_context(tc.tile_pool(name="pfin", bufs=1, space="PSUM"))

        ident = const_pool.tile([128, 128], F32)
        masks.make_identity(nc, ident[:])
        identr = const_pool.tile([128, 128], F32R)
        nc.vector.tensor_copy(identr[:], ident[:])

        # ---- Phase 0: h = x @ W^T ----
        w_sb = const_pool.tile([128, C], F32)
        nc.scalar.dma_start(out=w_sb[:], in_=w_d[:, :])
        psum_wt = psum_misc.tile([128, 128], F32, tag="pm")
        nc.tensor.transpose(psum_wt[:], w_sb[:], ident[:])
        wtr_sb = const_pool.tile([128, C], F32R)
        nc.vector.tensor_copy(wtr_sb[:], psum_wt[:])

        h_sb = h_pool.tile([128, NKT, C], F32R)
        XCH = 16  # x DMA chunks so h-compute overlaps the load
        NTX = NKT // XCH
        for xc in range(XCH):
            x_raw = xr_pool.tile([128, NTX, C], F32, tag="xraw")
            nc.scalar.dma_start(
                out=x_raw[:],
                in_=x_d[xc * NTX * 128 : (xc + 1) * NTX * 128, :].rearrange(
                    "(t p) c -> p t c", p=128
                ),
            )
            x_rnd = xr_pool.tile([128, NTX, C], F32R, tag="xrnd")
            nc.vector.tensor_copy(x_rnd[:], x_raw[:])  # fp32r rounding pass
            for tt in range(NTX):
                t = xc * NTX + tt
                psum_xt = psum_misc.tile([128, 128], F32R, tag="pm")
                nc.tensor.transpose(psum_xt[:], x_rnd[:, tt, :], identr[:])
                xt_sb = xt_pool.tile([128, 128], F32R)
                cp1 = nc.vector.tensor_copy if t % 2 == 0 else nc.scalar.copy
                cp2 = nc.scalar.copy if t % 2 == 0 else nc.vector.tensor_copy
                cp1(xt_sb[:], psum_xt[:])
                psum_h = psum_misc.tile([128, 128], F32, tag="pm")
                nc.tensor.matmul(psum_h[:], xt_sb[:], wtr_sb[:], start=True, stop=True)
                cp2(h_sb[:, t, :], psum_h[:])

        # h rows owned by this core (for the +I self-loop), exact fp32 path
        xo_sb = xr_pool.tile([128, R // 128, C], F32, tag="xo")
        nc.scalar.dma_start(
            out=xo_sb[:], in_=xo_d.rearrange("(t p) c -> p t c", p=128)
        )
        ho_sb = h_pool.tile([128, R // 128, C], F32)
        for t in range(R // 128):
            psum_xt = psum_misc.tile([128, 128], F32, tag="pm")
            nc.tensor.transpose(psum_xt[:], xo_sb[:, t, :], ident[:])
            xt_f = xt_pool.tile([128, 128], F32, tag="xtf")
            nc.vector.tensor_copy(xt_f[:], psum_xt[:])
            psum_h = psum_misc.tile([128, 128], F32, tag="pm")
            nc.tensor.matmul(
                psum_h[:], xt_f[:], wtr_sb[:].bitcast(F32), start=True, stop=True
            )
            nc.vector.tensor_copy(ho_sb[:, t, :], psum_h[:])

        # ---- Phase 1: main loop over (m-block, k-chunk) ----
        def do_chunk(blk, pacc, k0, kw):
            # load/multiply in half-chunks of 2 slabs: finer buffer
            # release keeps the DMA queue streaming without stalls
            quarters = []
            for qs in range(4):
                r0 = blk * M_BLK + qs * 128
                adj_t = adj_pool.tile([128, kw], F32, tag="adj")
                nc.sync.dma_start(
                    out=adj_t[:],
                    in_=adj_d[r0 : r0 + 128, k0 : k0 + kw].rearrange(
                        "(s p) k -> p (s k)", p=128
                    ),
                )
                mask_t = mask_pool.tile([128, kw], F32, tag="mask")
                nc.sync.dma_start(
                    out=mask_t[:],
                    in_=mask_d[r0 : r0 + 128, k0 : k0 + kw].rearrange(
                        "(s p) k -> p (s k)", p=128
                    ),
                )
                prod_t = prod_pool.tile([128, kw], F32R, tag="prod")
                nc.vector.tensor_mul(prod_t[:], adj_t[:], mask_t[:])
                quarters.append(prod_t)

            for kt in range(kw // 128):
                kg = (k0 // 128) + kt  # global k-tile index 0..63
                psum_at = psum_tr.tile([128, M_BLK], F32R)
                for s in range(S):
                    nc.tensor.transpose(
                        psum_at[:, s * 128 : (s + 1) * 128],
                        quarters[s][:, kt * 128 : (kt + 1) * 128],
                        identr[:],
                    )
                at_sb = at_pool.tile([128, M_BLK], F32R)
                nc.scalar.copy(at_sb[:], psum_at[:])
                nc.tensor.matmul(
                    pacc[:],
                    h_sb[:, kg, :],
                    at_sb[:],
                    start=(kg == 0),
                    stop=(kg == NKT - 1),
                )

        for blk in range(NBLK):
            pacc = psum_acc.tile([128, M_BLK], F32)
            for q in range(NQ):
                if blk == NBLK - 1 and q == NQ - 1:
                    # split the very last chunk: shorter dependency chain
                    # after the final bytes land -> smaller kernel tail
                    do_chunk(blk, pacc, q * KQ, KQ // 2)
                    do_chunk(blk, pacc, q * KQ + KQ // 2, KQ // 2)
                else:
                    do_chunk(blk, pacc, q * KQ, KQ)

            # ---- finalize m-block: back-transpose out^T, add self-loop h ----
            outT_sb = fin_pool.tile([128, M_BLK], F32)
            nc.vector.tensor_copy(outT_sb[:], pacc[:])
            psum_nat = psum_fin.tile([128, M_BLK], F32)
            for s in range(S):
                nc.tensor.transpose(
                    psum_nat[:, s * 128 : (s + 1) * 128],
                    outT_sb[:, s * 128 : (s + 1) * 128],
                    ident[:],
                )
            out_sb = fin_pool.tile([128, S, C], F32)
            nc.vector.tensor_add(
                out_sb[:],
                psum_nat[:].rearrange("p (s c) -> p s c", s=S),
                ho_sb[:, blk * S : (blk + 1) * S, :],
            )
            nc.scalar.dma_start(
                out=out_d[blk * M_BLK : (blk + 1) * M_BLK, :].rearrange(
                    "(s p) c -> p s c", p=128
                ),
                in_=out_sb[:],
            )

    nc.compile()
    return nc


_NC_CACHE = None


def _get_nc():
    global _NC_CACHE
    if _NC_CACHE is None:
        _NC_CACHE = build_program()
    return _NC_CACHE


def kernel(x, adj, mask, W):
    x = np.ascontiguousarray(x, dtype=np.float32)
    adj = np.ascontiguousarray(adj, dtype=np.float32)
    mask = np.ascontiguousarray(mask, dtype=np.float32)
    W = np.ascontiguousarray(W, dtype=np.float32)

    nc = _get_nc()
    in_maps = []
    for i in range(NCORES):
        r0 = i * R
        in_maps.append(
            {
                "adj": adj[r0 : r0 + R],
                "mask": mask[r0 : r0 + R],
                "x": x,
                "x_own": x[r0 : r0 + R],
                "w": W,
            }
        )
    res = run_bass_kernel_spmd(nc, in_maps, list(range(NCORES)))
    return np.concatenate([res.results[i]["out"] for i in range(NCORES)], axis=0)



# revision 3
# speedup vs baseline: 2.0788x; 2.0788x over previous
"""GCN layer kernel for Trainium2, 8-core row-parallel, fp16 streaming.

Computes out = (adj * mask + I) @ (x @ W^T) for N=8192, C_in=C_out=128.

Host-side sharding (inside kernel()): core i gets the row-block
adj[i*1024:(i+1)*1024] / mask rows, pre-TRANSPOSED and cast to fp16 so the
device needs no PE transposes at all; x / W are replicated as x^T / W^T
fp16. The device computes out^T = (A⊙M)^T-contracted row-block + h_own^T
and the host transposes back. fp16 inputs halve HBM traffic (the kernel is
memory-bound on the adj/mask stream); accumulation stays fp32 in PSUM, so
the end-to-end rel err is ~5e-4.

Per-core device pipeline:
  - h = x @ W^T via 64 k-tile matmuls: stationary = x^T k-slice (fp16,
    loaded directly), moving = W^T; psum f32 -> h_sb fp16.
  - hoT = W @ x_own^T (one stationary load) for the +I self-loop, kept f32.
  - main loop kg=0..63: one DMA loads [128, 2048] = adj^T k-tile ‖ mask^T
    k-tile (4KB descriptors), DVE multiplies them (fp16, 2x rate), two
    matmuls accumulate outT[:, :512] / outT[:, 512:] in PSUM across all 64
    k-tiles (stationary = h k-tile, moving = product).
  - last k-tile is split into m-halves so the tail after the final bytes is
    just mul -> matmul -> add -> store of one 256KB half.
  - finalize: outT = pacc + hoT on DVE, stored straight from SBUF.
"""

import numpy as np
from contextlib import ExitStack

from concourse import bass, bacc, tile, mybir
from concourse.bass_utils import run_bass_kernel_spmd

N = 8192
C = 128
NCORES = 8
R = N // NCORES          # 1024 rows per core
NKT = N // 128           # 64 k-tiles
XCH = 4                  # x^T DMA chunks
XW = N // XCH            # 2048 columns per x^T chunk

F32 = mybir.dt.float32
F16 = mybir.dt.float16


def build_program():
    nc = bacc.Bacc("TRN2", target_bir_lowering=False, debug=False, num_devices=NCORES)

    ab_d = nc.dram_tensor("ab", [N, 2 * R], F16, kind="ExternalInput").ap()
    xt_d = nc.dram_tensor("xt", [C, N], F16, kind="ExternalInput").ap()
    xot_d = nc.dram_tensor("xot", [C, R], F16, kind="ExternalInput").ap()
    wt_d = nc.dram_tensor("wt", [C, C], F16, kind="ExternalInput").ap()
    out_d = nc.dram_tensor("out", [C, R], F32, kind="ExternalOutput").ap()

    with tile.TileContext(nc) as tc, ExitStack() as ctx:
        const_pool = ctx.enter_context(tc.tile_pool(name="const", bufs=1))
        xt_pool = ctx.enter_context(tc.tile_pool(name="xt", bufs=XCH))
        h_pool = ctx.enter_context(tc.tile_pool(name="h", bufs=1))
        ab_pool = ctx.enter_context(tc.tile_pool(name="ab", bufs=10))
        prod_pool = ctx.enter_context(tc.tile_pool(name="prod", bufs=6))
        fin_pool = ctx.enter_context(tc.tile_pool(name="fin", bufs=2))
        psum_acc = ctx.enter_context(tc.tile_pool(name="pacc", bufs=2, space="PSUM"))
        psum_h = ctx.enter_context(tc.tile_pool(name="ph", bufs=2, space="PSUM"))
        psum_hot = ctx.enter_context(tc.tile_pool(name="phot", bufs=2, space="PSUM"))

        # ---- header DMAs (scalar/ACT queue) ----
        wt_sb = const_pool.tile([C, C], F16)
        nc.scalar.dma_start(out=wt_sb[:], in_=wt_d[:, :])
        xt_sb = []
        for xc in range(XCH):
            t = xt_pool.tile([C, XW], F16, tag="xt")
            nc.scalar.dma_start(out=t[:], in_=xt_d[:, xc * XW : (xc + 1) * XW])
            xt_sb.append(t)
        xot_sb = const_pool.tile([C, R], F16)
        nc.scalar.dma_start(out=xot_sb[:], in_=xot_d[:, :])

        # ---- AB stream (sync/SP queue): adj^T‖mask^T k-tiles ----
        ab_t = []
        for kg in range(NKT - 1):
            t = ab_pool.tile([128, 2 * R], F16, tag="ab")
            nc.sync.dma_start(out=t[:], in_=ab_d[kg * 128 : (kg + 1) * 128, :])
            ab_t.append(t)
        # last k-tile split into m-halves (tail shortening): 4 loads
        k0 = (NKT - 1) * 128
        last = []
        for half in range(2):
            ta = ab_pool.tile([128, 512], F16, tag="ab")
            nc.sync.dma_start(
                out=ta[:], in_=ab_d[k0 : k0 + 128, half * 512 : half * 512 + 512]
            )
            tm = ab_pool.tile([128, 512], F16, tag="ab")
            nc.sync.dma_start(
                out=tm[:],
                in_=ab_d[k0 : k0 + 128, R + half * 512 : R + half * 512 + 512],
            )
            last.append((ta, tm))

        # ---- phase 0: h = x @ W^T (PE + ACT copies) ----
        h_sb = h_pool.tile([128, NKT, C], F16)
        for kg in range(NKT):
            ph = psum_h.tile([128, C], F32, tag="ph")
            nc.tensor.matmul(
                ph[:],
                xt_sb[kg // (XW // 128)][:, (kg % (XW // 128)) * 128 : (kg % (XW // 128)) * 128 + 128],
                wt_sb[:],
                start=True,
                stop=True,
            )
            nc.scalar.copy(h_sb[:, kg, :], ph[:])

        # hoT = W @ x_own^T, kept f32 for the self-loop add
        hot_sb = const_pool.tile([C, R], F32)
        for half in range(2):
            phot = psum_hot.tile([C, 512], F32, tag="phot")
            nc.tensor.matmul(
                phot[:],
                wt_sb[:],
                xot_sb[:, half * 512 : half * 512 + 512],
                start=True,
                stop=True,
            )
            nc.scalar.copy(hot_sb[:, half * 512 : half * 512 + 512], phot[:])

        # ---- main loop: accumulate outT over 64 k-tiles ----
        pacc0 = psum_acc.tile([128, 512], F32, tag="pacc")
        pacc1 = psum_acc.tile([128, 512], F32, tag="pacc")
        pacc = [pacc0, pacc1]
        for kg in range(NKT - 1):
            prod = prod_pool.tile([128, R], F16, tag="prod")
            nc.vector.tensor_mul(prod[:], ab_t[kg][:, :R], ab_t[kg][:, R:])
            for half in range(2):
                nc.tensor.matmul(
                    pacc[half][:],
                    h_sb[:, kg, :],
                    prod[:, half * 512 : half * 512 + 512],
                    start=(kg == 0),
                    stop=False,
                )
        # last k-tile: per-half mul+matmul, then finalize that half
        outT = fin_pool.tile([C, R], F32)
        for half in range(2):
            ta, tm = last[half]
            prod = prod_pool.tile([128, 512], F16, tag="prod")
            nc.vector.tensor_mul(prod[:], ta[:], tm[:])
            nc.tensor.matmul(
                pacc[half][:], h_sb[:, NKT - 1, :], prod[:], start=False, stop=True
            )
            nc.vector.tensor_add(
                outT[:, half * 512 : half * 512 + 512],
                pacc[half][:],
                hot_sb[:, half * 512 : half * 512 + 512],
            )
            nc.scalar.dma_start(
                out=out_d[:, half * 512 : half * 512 + 512],
                in_=outT[:, half * 512 : half * 512 + 512],
            )

    nc.compile()
    return nc


_NC_CACHE = None


def _get_nc():
    global _NC_CACHE
    if _NC_CACHE is None:
        _NC_CACHE = build_program()
    return _NC_CACHE


def make_in_maps(x, adj, mask, W):
    """Host-side sharding: per-core transposed fp16 inputs."""
    xt = np.ascontiguousarray(x.T, dtype=np.float16)
    wt = np.ascontiguousarray(W.T, dtype=np.float16)
    in_maps = []
    for i in range(NCORES):
        r0 = i * R
        ab = np.empty((N, 2 * R), dtype=np.float16)
        ab[:, :R] = adj[r0 : r0 + R].T
        ab[:, R:] = mask[r0 : r0 + R].T
        in_maps.append(
            {
                "ab": ab,
                "xt": xt,
                "xot": np.ascontiguousarray(xt[:, r0 : r0 + R]),
                "wt": wt,
            }
        )
    return in_maps


def kernel(x, adj, mask, W):
    x = np.asarray(x, dtype=np.float32)
    adj = np.asarray(adj, dtype=np.float32)
    mask = np.asarray(mask, dtype=np.float32)
    W = np.asarray(W, dtype=np.float32)

    nc = _get_nc()
    res = run_bass_kernel_spmd(nc, make_in_maps(x, adj, mask, W), list(range(NCORES)))
    out = np.empty((N, C), dtype=np.float32)
    for i in range(NCORES):
        out[i * R : (i + 1) * R] = res.results[i]["out"].T
    return out
